# revision 21
# baseline (speedup 1.0000x reference)
"""Single-head causal attention (B=8, T=2048, D=1024, fp32 I/O) on 8 trn2
NeuronCores, data-parallel over batch (one batch element per core).

Per-core algorithm (all matmuls bf16 with fp32 PSUM accumulation):
  xT   = transpose(cast_bf16(x))                  via PE-transpose, pipelined
  qT   = Wq^T-stationary matmuls  -> (e, t) layout
  kT   = same                                     -> (e, t) layout
  v    = xT-stationary matmuls    -> (t, e) layout
  S^T  block (j, i) = kT(:,j)-stationary @ qT     (contraction over e)
  E^T  = exp(S^T / 32)   (no max-subtraction needed: |S/32| <~ 2)
         diagonal blocks masked by an upper-triangular 0/1 multiply
  rowsum_i = ones-matmul with E^T stationary      (PSUM accumulation over j)
  out  = (E^T-stationary @ v) * (1/rowsum)        per-partition scalar scale

The softmax normalization is applied to the AV output instead of to the
weights, so no transposes of the (T, T) attention matrix are ever needed.
Score spans are aligned to the causal diagonal so no masked block is ever
computed except the triangular diagonal blocks themselves.
"""
import sys
import types

import numpy as np

import concourse.bass as bass
import concourse.mybir as mybir
import concourse.tile as tile
from concourse.bass_utils import run_bass_kernel_spmd
from concourse.masks import make_identity, make_upper_triangular

B, T, D = 8, 2048, 1024
P = 128
TB = T // P        # 16 t-blocks
DBLK = D // P      # 8 d/e-blocks
NTS = T // 512     # 4 t-spans of 512
NES = D // 512     # 2 e-spans of 512
SCALE = 1.0 / 32.0  # 1/sqrt(D)

F32 = mybir.dt.float32
BF16 = mybir.dt.bfloat16


def _install_ntff_hook():
    """Optional: register the axon NTFF profiling hook (the agent image's
    antenv lacks axon_hooks). Lets BASS_TRACE=1 produce exec_time_ns."""
    try:
        import antenv

        if "antenv.axon_hooks" in sys.modules:
            return
        mod = types.ModuleType("antenv.axon_hooks")
        _hook = [None]
        mod.set_axon_ntff_profile_hook = lambda h: _hook.__setitem__(0, h)
        mod.get_axon_ntff_profile_hook = lambda: _hook[0]
        sys.modules["antenv.axon_hooks"] = mod
        antenv.axon_hooks = mod
        from trn_agent_boot.trn_boot import _ntff_profile_via_ctypes

        mod.set_axon_ntff_profile_hook(
            _ntff_profile_via_ctypes("/opt/axon/libaxon_pjrt.so")
        )
    except Exception:
        pass


_install_ntff_hook()


def _split_multi_waits(nc: bass.Bass):
    """Walrus on this stack fits only ONE sync-wait per instruction, but
    Tile emits several on multi-producer instructions. Hoist the extra waits
    onto single-wait NoOps placed just before, on the same engine — the
    per-engine streams are in-order, so semantics are identical."""
    n_split = 0
    for fn in nc.m.functions:
        for bb in fn.blocks:
            out = []
            changed = False
            for inst in bb.instructions:
                si = inst.sync_info
                waits = list(si.on_wait) if si is not None and si.on_wait else []
                if len(waits) > 1:
                    for w in waits[:-1]:
                        nop = mybir.InstNoOp(
                            name=nc.get_next_instruction_name(),
                            engine=inst.engine,
                            ins=[],
                            outs=[],
                            sync_info=mybir.SyncInfo(on_wait=[w], on_update=[]),
                            bass_nofuse=True,
                        )
                        out.append(nop)
                    inst.sync_info = mybir.SyncInfo(
                        on_wait=[waits[-1]],
                        on_update=list(si.on_update or []),
                    )
                    changed = True
                    n_split += 1
                out.append(inst)
            if changed:
                bb.instructions = out
    return n_split


def _emit(nc: bass.Bass):
    x = nc.dram_tensor("x", [T, D], F32, kind="ExternalInput").ap()
    Wq = nc.dram_tensor("Wq", [D, D], F32, kind="ExternalInput").ap()
    Wk = nc.dram_tensor("Wk", [D, D], F32, kind="ExternalInput").ap()
    Wv = nc.dram_tensor("Wv", [D, D], F32, kind="ExternalInput").ap()
    out = nc.dram_tensor("out", [T, D], F32, kind="ExternalOutput").ap()

    with tile.TileContext(nc) as tc:
        from contextlib import ExitStack

        with ExitStack() as ctx:
            persist = ctx.enter_context(tc.tile_pool(name="persist", bufs=1))
            psum = ctx.enter_context(tc.tile_pool(name="psum", bufs=6, space="PSUM"))

            # ---- persistent SBUF tensors (survive the whole kernel) ----
            qT = persist.tile([P, DBLK, T], BF16)       # (e, t)
            kT = persist.tile([P, DBLK, T], BF16)       # (e, t)
            vsb = persist.tile([P, TB, D], BF16)        # (t, e)
            # E^T tiles for i-spans 0-1 (computed early, inside phase B,
            # to fill the PE bubble while ts2/ts3 xT transposes land)
            etE = persist.tile([P, 12, 512], BF16)      # (j, i) blocks
            ones = persist.tile([P, 1], BF16)
            triu = persist.tile([P, P], BF16)
            ident = persist.tile([P, P], BF16)

            nc.vector.memset(ones, 1.0)
            # triu[j, i] = 1 where i >= j (keep), 0 below the diagonal.
            make_upper_triangular(nc, triu, val=1.0, diag=True)
            make_identity(nc, ident)

            # ============ Phase A+B: load/cast/transpose + QKV ==========
            dram = ctx.enter_context(tc.tile_pool(name="dram", bufs=1, space="DRAM"))
            with tc.tile_pool(name="qkvp", bufs=1) as qkvp, \
                 tc.tile_pool(name="staging", bufs=4) as staging:
                xT = qkvp.tile([P, DBLK, T], BF16)          # (d, t)
                # Wq/Wk/Wv share two 8KB slots: Wv reuses Wq's slot after
                # the last q matmul has read it.
                wq_bf = qkvp.tile([P, DBLK, D], BF16, tag="wbf", bufs=2)
                wk_bf = qkvp.tile([P, DBLK, D], BF16, tag="wbf", bufs=2)
                wv_bf = qkvp.tile([P, DBLK, D], BF16, tag="wbf", bufs=2)

                xbf_dram = dram.tile([T, D], BF16)

                def x_chain_pe(tb):
                    """Load x row-block tb, cast to bf16, PE-transpose the
                    8 [128,128] sub-blocks into xT (fast path for the first
                    t-span, before the xbar pipeline has warmed up)."""
                    stage_f32 = staging.tile([P, D], F32, tag="xs32", bufs=4)
                    nc.sync.dma_start(
                        out=stage_f32, in_=x[tb * P:(tb + 1) * P, :]
                    )
                    stage_bf = staging.tile([P, D], BF16, tag="xsbf", bufs=3)
                    nc.vector.tensor_copy(stage_bf, stage_f32)
                    for db in range(DBLK):
                        pst = psum.tile([P, P], BF16, tag="small", bufs=2)
                        nc.tensor.transpose(
                            pst, stage_bf[:, db * P:(db + 1) * P], ident
                        )
                        nc.any.tensor_copy(xT[:, db, tb * P:(tb + 1) * P], pst)

                def x_chain_store(tb):
                    """Load x row-block tb, cast to bf16, store to DRAM for
                    the xbar-transposed reload (keeps the PE free)."""
                    stage_f32 = staging.tile([P, D], F32, tag="xs32", bufs=4)
                    nc.sync.dma_start(
                        out=stage_f32, in_=x[tb * P:(tb + 1) * P, :]
                    )
                    stage_bf = staging.tile([P, D], BF16, tag="xsbf", bufs=3)
                    nc.vector.tensor_copy(stage_bf, stage_f32)
                    nc.sync.dma_start(
                        out=xbf_dram[tb * P:(tb + 1) * P, :], in_=stage_bf
                    )

                filler_q = []

                def x_store_deferred(tb):
                    """Load now; enqueue the cast (4 pieces, so they slot
                    between B-phase PSUM copies without blocking them) and
                    the DRAM store as filler actions drained inside B."""
                    stage_f32 = staging.tile([P, D], F32, tag="xs32", bufs=4)
                    nc.sync.dma_start(
                        out=stage_f32, in_=x[tb * P:(tb + 1) * P, :]
                    )
                    stage_bf = staging.tile([P, D], BF16, tag="xsbf", bufs=3)

                    def piece(pc, f32=stage_f32, bf=stage_bf):
                        nc.vector.tensor_copy(
                            bf[:, pc * 256:(pc + 1) * 256],
                            f32[:, pc * 256:(pc + 1) * 256],
                        )

                    def store(bf=stage_bf, tb=tb):
                        nc.sync.dma_start(
                            out=xbf_dram[tb * P:(tb + 1) * P, :], in_=bf
                        )

                    for pc in range(4):
                        filler_q.append(lambda pc=pc: piece(pc))
                    filler_q.append(store)

                def drain_filler(n):
                    for _ in range(n):
                        if filler_q:
                            filler_q.pop(0)()

                def xbar_batch(ts):
                    for db in range(DBLK):
                        nc.sync.dma_start_transpose(
                            out=xT[:, db, ts * 512:(ts + 1) * 512],
                            in_=xbf_dram[ts * 512:(ts + 1) * 512,
                                         db * P:(db + 1) * P],
                        )

                wcast = [0]

                def w_chain(w_dram, w_sb, db, dma_engine=None, defer=False):
                    """Casts alternate DVE/ACT so neither paces the stream.
                    With defer=True the cast becomes a filler action."""
                    stage_f32 = staging.tile([P, D], F32, tag="ws32", bufs=3)
                    (dma_engine or nc.sync).dma_start(
                        out=stage_f32, in_=w_dram[db * P:(db + 1) * P, :]
                    )

                    def cast(f32=stage_f32):
                        if wcast[0] % 2 == 0:
                            nc.vector.tensor_copy(w_sb[:, db, :], f32)
                        else:
                            nc.scalar.copy(w_sb[:, db, :], f32)
                        wcast[0] += 1

                    if defer:
                        filler_q.append(cast)
                    else:
                        cast()

                def score_exp(jb, i0, L, et_ap):
                    """S^T block row jb over i in [i0, i0+L): matmul,
                    exp (scaled), diagonal mask if the span starts on the
                    causal diagonal."""
                    ps = psum.tile([P, 512], F32, tag="big")
                    for eb in range(DBLK):
                        nc.tensor.matmul(
                            ps[:, 0:L],
                            lhsT=kT[:, eb, jb * P:(jb + 1) * P],
                            rhs=qT[:, eb, i0:i0 + L],
                            start=(eb == 0),
                            stop=(eb == DBLK - 1),
                        )
                    nc.scalar.activation(
                        et_ap, ps[:, 0:L],
                        mybir.ActivationFunctionType.Exp, scale=SCALE,
                    )
                    if jb * P >= i0:  # diagonal block leads this span
                        nc.vector.tensor_mul(
                            et_ap[:, 0:P], et_ap[:, 0:P], triu
                        )

                def qk_group(w_sb, dstT, ts, fill=0):
                    for eb in range(DBLK):
                        ps = psum.tile([P, 512], F32, tag="big")
                        for db in range(DBLK):
                            nc.tensor.matmul(
                                ps,
                                lhsT=w_sb[:, db, eb * P:(eb + 1) * P],
                                rhs=xT[:, db, ts * 512:(ts + 1) * 512],
                                start=(db == 0),
                                stop=(db == DBLK - 1),
                            )
                        nc.any.tensor_copy(
                            dstT[:, eb, ts * 512:(ts + 1) * 512], ps
                        )
                        drain_filler(fill)

                # DMA delivery order matches PE consumption order; late x
                # tiles are staged between B groups so their DVE casts never
                # block earlier PSUM evacuations in the static engine order.
                for tb in range(8):
                    x_chain_pe(tb)
                for db in range(DBLK):
                    w_chain(Wq, wq_bf, db)
                for db in range(DBLK):
                    w_chain(Wk, wk_bf, db)
                qk_group(wq_bf, qT, 0)
                x_chain_store(8)
                x_chain_store(9)
                qk_group(wq_bf, qT, 1)
                x_chain_store(10)
                x_chain_store(11)
                qk_group(wk_bf, kT, 0)
                x_chain_store(12)
                x_chain_store(13)
                qk_group(wk_bf, kT, 1)
                x_chain_store(14)
                x_chain_store(15)
                xbar_batch(2)
                # Early scores for i-spans 0-1: fills the PE bubble while
                # the ts2/ts3 xbar transposes complete.
                eidx = 0
                for s in range(2):
                    for jb in range(4 * s + 4):
                        i0 = max(s * 512, jb * P)
                        L = (s + 1) * 512 - i0
                        score_exp(jb, i0, L, etE[:, eidx, 0:L])
                        eidx += 1
                xbar_batch(3)
                qk_group(wq_bf, qT, 2)
                qk_group(wk_bf, kT, 2)
                qk_group(wq_bf, qT, 3)
                qk_group(wk_bf, kT, 3)

                # v: out[t(128), e(512)] = sum_d xT[d, t]-stat @ W[d, e]
                for db in range(DBLK):
                    w_chain(Wv, wv_bf, db, dma_engine=nc.gpsimd)
                for tb in range(TB):
                    for es in range(NES):
                        ps = psum.tile([P, 512], F32, tag="big")
                        for db in range(DBLK):
                            nc.tensor.matmul(
                                ps,
                                lhsT=xT[:, db, tb * P:(tb + 1) * P],
                                rhs=wv_bf[:, db, es * 512:(es + 1) * 512],
                                start=(db == 0),
                                stop=(db == DBLK - 1),
                            )
                        nc.any.tensor_copy(vsb[:, tb, es * 512:(es + 1) * 512], ps)

            # ================= Phase C+D: attention =====================
            with tc.tile_pool(name="etp", bufs=16) as etp, \
                 tc.tile_pool(name="outp", bufs=4) as outp, \
                 tc.tile_pool(name="rsp", bufs=4) as rsp:
                eidx = 0
                for s in range(NTS):
                    # --- scores + exp for i-span s, all jb <= 4s+3 ---
                    # (spans 0-1 were already computed inside phase B; see
                    # the early-scores fill)
                    et_tiles = []
                    et_i0 = []
                    for jb in range(4 * s + 4):
                        i0 = max(s * 512, jb * P)
                        L = (s + 1) * 512 - i0
                        if s < 2:
                            et = etE[:, eidx, :]
                            eidx += 1
                        else:
                            et = etp.tile([P, 512], BF16, tag="et")
                            score_exp(jb, i0, L, et[:, 0:L])
                        et_tiles.append(et)
                        et_i0.append(i0)

                    # --- AV + rowsums for the 4 i-blocks in span s ---
                    for ib in range(4 * s, 4 * s + 4):
                        ps0 = psum.tile([P, 512], F32, tag="big")
                        ps1 = psum.tile([P, 512], F32, tag="big")
                        pss = psum.tile([P, 1], F32, tag="small", bufs=2)
                        for jb in range(ib + 1):
                            off = ib * P - et_i0[jb]
                            lhsT = et_tiles[jb][:, off:off + P]
                            first = jb == 0
                            last = jb == ib
                            nc.tensor.matmul(
                                ps0, lhsT=lhsT, rhs=vsb[:, jb, 0:512],
                                start=first, stop=last,
                            )
                            nc.tensor.matmul(
                                ps1, lhsT=lhsT, rhs=vsb[:, jb, 512:1024],
                                start=first, stop=last,
                            )
                            nc.tensor.matmul(
                                pss, lhsT=lhsT, rhs=ones,
                                start=first, stop=last,
                            )
                        rsum = rsp.tile([P, 1], F32)
                        nc.vector.reciprocal(rsum, pss)
                        for es, ps in ((0, ps0), (1, ps1)):
                            ob = outp.tile([P, 512], F32)
                            nc.vector.tensor_scalar_mul(ob, ps, rsum)
                            nc.sync.dma_start(
                                out=out[ib * P:(ib + 1) * P,
                                        es * 512:(es + 1) * 512],
                                in_=ob,
                            )
    return nc


_NC_CACHE = None


def _get_nc():
    global _NC_CACHE
    if _NC_CACHE is None:
        nc = bass.Bass(
            "TRN2", target_bir_lowering=False, debug=False, num_devices=1
        )
        _emit(nc)
        _split_multi_waits(nc)
        _NC_CACHE = nc
    return _NC_CACHE


def kernel(x, Wq, Wk, Wv):
    assert x.shape == (B, T, D), x.shape
    nc = _get_nc()
    Wq = np.ascontiguousarray(Wq, dtype=np.float32)
    Wk = np.ascontiguousarray(Wk, dtype=np.float32)
    Wv = np.ascontiguousarray(Wv, dtype=np.float32)
    in_maps = [
        {
            "x": np.ascontiguousarray(x[b], dtype=np.float32),
            "Wq": Wq,
            "Wk": Wk,
            "Wv": Wv,
        }
        for b in range(B)
    ]
    res = run_bass_kernel_spmd(nc, in_maps, core_ids=list(range(B)))
    out = np.stack([res.results[b]["out"] for b in range(B)], axis=0)
    kernel.last_exec_time_ns = res.exec_time_ns
    return out


# revision 22
# speedup vs baseline: 1.0187x; 1.0187x over previous
"""Single-head causal attention (B=8, T=2048, D=1024, fp32 I/O) on 8 trn2
NeuronCores, data-parallel over batch (one batch element per core).

Per-core algorithm (all matmuls bf16 with fp32 PSUM accumulation):
  xT   = transpose(cast_bf16(x))                  via PE-transpose, pipelined
  qT   = Wq^T-stationary matmuls  -> (e, t) layout
  kT   = same                                     -> (e, t) layout
  v    = xT-stationary matmuls    -> (t, e) layout
  S^T  block (j, i) = kT(:,j)-stationary @ qT     (contraction over e)
  E^T  = exp(S^T / 32)   (no max-subtraction needed: |S/32| <~ 2)
         diagonal blocks masked by an upper-triangular 0/1 multiply
  rowsum_i = ones-matmul with E^T stationary      (PSUM accumulation over j)
  out  = (E^T-stationary @ v) * (1/rowsum)        per-partition scalar scale

The softmax normalization is applied to the AV output instead of to the
weights, so no transposes of the (T, T) attention matrix are ever needed.
Score spans are aligned to the causal diagonal so no masked block is ever
computed except the triangular diagonal blocks themselves.
"""
import sys
import types

import numpy as np

import concourse.bass as bass
import concourse.mybir as mybir
import concourse.tile as tile
from concourse.bass_utils import run_bass_kernel_spmd
from concourse.masks import make_identity, make_upper_triangular

B, T, D = 8, 2048, 1024
P = 128
TB = T // P        # 16 t-blocks
DBLK = D // P      # 8 d/e-blocks
NTS = T // 512     # 4 t-spans of 512
NES = D // 512     # 2 e-spans of 512
SCALE = 1.0 / 32.0  # 1/sqrt(D)

F32 = mybir.dt.float32
BF16 = mybir.dt.bfloat16


def _install_ntff_hook():
    """Optional: register the axon NTFF profiling hook (the agent image's
    antenv lacks axon_hooks). Lets BASS_TRACE=1 produce exec_time_ns."""
    try:
        import antenv

        if "antenv.axon_hooks" in sys.modules:
            return
        mod = types.ModuleType("antenv.axon_hooks")
        _hook = [None]
        mod.set_axon_ntff_profile_hook = lambda h: _hook.__setitem__(0, h)
        mod.get_axon_ntff_profile_hook = lambda: _hook[0]
        sys.modules["antenv.axon_hooks"] = mod
        antenv.axon_hooks = mod
        from trn_agent_boot.trn_boot import _ntff_profile_via_ctypes

        mod.set_axon_ntff_profile_hook(
            _ntff_profile_via_ctypes("/opt/axon/libaxon_pjrt.so")
        )
    except Exception:
        pass


_install_ntff_hook()


def _split_multi_waits(nc: bass.Bass):
    """Walrus on this stack fits only ONE sync-wait per instruction, but
    Tile emits several on multi-producer instructions. Hoist the extra waits
    onto single-wait NoOps placed just before, on the same engine — the
    per-engine streams are in-order, so semantics are identical."""
    n_split = 0
    for fn in nc.m.functions:
        for bb in fn.blocks:
            out = []
            changed = False
            for inst in bb.instructions:
                si = inst.sync_info
                waits = list(si.on_wait) if si is not None and si.on_wait else []
                if len(waits) > 1:
                    for w in waits[:-1]:
                        nop = mybir.InstNoOp(
                            name=nc.get_next_instruction_name(),
                            engine=inst.engine,
                            ins=[],
                            outs=[],
                            sync_info=mybir.SyncInfo(on_wait=[w], on_update=[]),
                            bass_nofuse=True,
                        )
                        out.append(nop)
                    inst.sync_info = mybir.SyncInfo(
                        on_wait=[waits[-1]],
                        on_update=list(si.on_update or []),
                    )
                    changed = True
                    n_split += 1
                out.append(inst)
            if changed:
                bb.instructions = out
    return n_split


def _emit(nc: bass.Bass):
    x = nc.dram_tensor("x", [T, D], F32, kind="ExternalInput").ap()
    Wq = nc.dram_tensor("Wq", [D, D], F32, kind="ExternalInput").ap()
    Wk = nc.dram_tensor("Wk", [D, D], F32, kind="ExternalInput").ap()
    Wv = nc.dram_tensor("Wv", [D, D], F32, kind="ExternalInput").ap()
    out = nc.dram_tensor("out", [T, D], F32, kind="ExternalOutput").ap()

    with tile.TileContext(nc) as tc:
        from contextlib import ExitStack

        with ExitStack() as ctx:
            persist = ctx.enter_context(tc.tile_pool(name="persist", bufs=1))
            psum = ctx.enter_context(tc.tile_pool(name="psum", bufs=6, space="PSUM"))

            # ---- persistent SBUF tensors (survive the whole kernel) ----
            qT = persist.tile([P, DBLK, T], BF16)       # (e, t)
            kT = persist.tile([P, DBLK, T], BF16)       # (e, t)
            vsb = persist.tile([P, TB, D], BF16)        # (t, e)
            # E^T tiles for i-spans 0-1 (computed early, inside phase B,
            # to fill the PE bubble while ts2/ts3 xT transposes land)
            etE = persist.tile([P, 12, 512], BF16)      # (j, i) blocks
            ones = persist.tile([P, 1], BF16)
            triu = persist.tile([P, P], BF16)
            ident = persist.tile([P, P], BF16)

            nc.vector.memset(ones, 1.0)
            # triu[j, i] = 1 where i >= j (keep), 0 below the diagonal.
            make_upper_triangular(nc, triu, val=1.0, diag=True)
            make_identity(nc, ident)

            # ============ Phase A+B: load/cast/transpose + QKV ==========
            dram = ctx.enter_context(tc.tile_pool(name="dram", bufs=1, space="DRAM"))
            with tc.tile_pool(name="qkvp", bufs=1) as qkvp, \
                 tc.tile_pool(name="staging", bufs=4) as staging:
                xT = qkvp.tile([P, DBLK, T], BF16)          # (d, t)
                # Wq/Wk/Wv share two 8KB slots: Wv reuses Wq's slot after
                # the last q matmul has read it.
                wq_bf = qkvp.tile([P, DBLK, D], BF16, tag="wbf", bufs=2)
                wk_bf = qkvp.tile([P, DBLK, D], BF16, tag="wbf", bufs=2)
                wv_bf = qkvp.tile([P, DBLK, D], BF16, tag="wbf", bufs=2)

                xbf_dram = dram.tile([T, D], BF16)

                def x_chain_pe(tb):
                    """Load x row-block tb, cast to bf16, PE-transpose the
                    8 [128,128] sub-blocks into xT (fast path for the first
                    t-span, before the xbar pipeline has warmed up)."""
                    stage_f32 = staging.tile([P, D], F32, tag="xs32", bufs=4)
                    nc.sync.dma_start(
                        out=stage_f32, in_=x[tb * P:(tb + 1) * P, :]
                    )
                    stage_bf = staging.tile([P, D], BF16, tag="xsbf", bufs=3)
                    nc.vector.tensor_copy(stage_bf, stage_f32)
                    for db in range(DBLK):
                        pst = psum.tile([P, P], BF16, tag="small", bufs=2)
                        nc.tensor.transpose(
                            pst, stage_bf[:, db * P:(db + 1) * P], ident
                        )
                        nc.any.tensor_copy(xT[:, db, tb * P:(tb + 1) * P], pst)

                def x_chain_store(tb):
                    """Load x row-block tb, cast to bf16, store to DRAM for
                    the xbar-transposed reload (keeps the PE free)."""
                    stage_f32 = staging.tile([P, D], F32, tag="xs32", bufs=4)
                    nc.sync.dma_start(
                        out=stage_f32, in_=x[tb * P:(tb + 1) * P, :]
                    )
                    stage_bf = staging.tile([P, D], BF16, tag="xsbf", bufs=3)
                    nc.vector.tensor_copy(stage_bf, stage_f32)
                    nc.sync.dma_start(
                        out=xbf_dram[tb * P:(tb + 1) * P, :], in_=stage_bf
                    )

                filler_q = []

                def x_store_deferred(tb):
                    """Load now; enqueue the cast (4 pieces, so they slot
                    between B-phase PSUM copies without blocking them) and
                    the DRAM store as filler actions drained inside B."""
                    stage_f32 = staging.tile([P, D], F32, tag="xs32", bufs=4)
                    nc.sync.dma_start(
                        out=stage_f32, in_=x[tb * P:(tb + 1) * P, :]
                    )
                    stage_bf = staging.tile([P, D], BF16, tag="xsbf", bufs=3)

                    def piece(pc, f32=stage_f32, bf=stage_bf):
                        nc.vector.tensor_copy(
                            bf[:, pc * 256:(pc + 1) * 256],
                            f32[:, pc * 256:(pc + 1) * 256],
                        )

                    def store(bf=stage_bf, tb=tb):
                        nc.sync.dma_start(
                            out=xbf_dram[tb * P:(tb + 1) * P, :], in_=bf
                        )

                    for pc in range(4):
                        filler_q.append(lambda pc=pc: piece(pc))
                    filler_q.append(store)

                def drain_filler(n):
                    for _ in range(n):
                        if filler_q:
                            filler_q.pop(0)()

                def xbar_batch(ts):
                    for db in range(DBLK):
                        nc.sync.dma_start_transpose(
                            out=xT[:, db, ts * 512:(ts + 1) * 512],
                            in_=xbf_dram[ts * 512:(ts + 1) * 512,
                                         db * P:(db + 1) * P],
                        )

                wcast = [0]

                def w_chain(w_dram, w_sb, db, dma_engine=None, defer=False):
                    """Casts alternate DVE/ACT so neither paces the stream.
                    With defer=True the cast becomes a filler action."""
                    stage_f32 = staging.tile([P, D], F32, tag="ws32", bufs=3)
                    (dma_engine or nc.sync).dma_start(
                        out=stage_f32, in_=w_dram[db * P:(db + 1) * P, :]
                    )

                    def cast(f32=stage_f32):
                        if wcast[0] % 2 == 0:
                            nc.vector.tensor_copy(w_sb[:, db, :], f32)
                        else:
                            nc.scalar.copy(w_sb[:, db, :], f32)
                        wcast[0] += 1

                    if defer:
                        filler_q.append(cast)
                    else:
                        cast()

                def score_exp(jb, i0, L, et_ap):
                    """S^T block row jb over i in [i0, i0+L): matmul,
                    exp (scaled), diagonal mask if the span starts on the
                    causal diagonal."""
                    ps = psum.tile([P, 512], F32, tag="big")
                    for eb in range(DBLK):
                        nc.tensor.matmul(
                            ps[:, 0:L],
                            lhsT=kT[:, eb, jb * P:(jb + 1) * P],
                            rhs=qT[:, eb, i0:i0 + L],
                            start=(eb == 0),
                            stop=(eb == DBLK - 1),
                        )
                    nc.scalar.activation(
                        et_ap, ps[:, 0:L],
                        mybir.ActivationFunctionType.Exp, scale=SCALE,
                    )
                    if jb * P >= i0:  # diagonal block leads this span
                        nc.vector.tensor_mul(
                            et_ap[:, 0:P], et_ap[:, 0:P], triu
                        )

                def qk_group(w_sb, dstT, ts, fill=0):
                    for eb in range(DBLK):
                        ps = psum.tile([P, 512], F32, tag="big")
                        for db in range(DBLK):
                            nc.tensor.matmul(
                                ps,
                                lhsT=w_sb[:, db, eb * P:(eb + 1) * P],
                                rhs=xT[:, db, ts * 512:(ts + 1) * 512],
                                start=(db == 0),
                                stop=(db == DBLK - 1),
                            )
                        nc.any.tensor_copy(
                            dstT[:, eb, ts * 512:(ts + 1) * 512], ps
                        )
                        drain_filler(fill)

                # DMA delivery order matches PE consumption order; late x
                # tiles are staged between B groups so their DVE casts never
                # block earlier PSUM evacuations in the static engine order.
                for i in range(8):
                    x_chain_pe(i)
                    w_chain(Wq, wq_bf, i)
                for db in range(DBLK):
                    w_chain(Wk, wk_bf, db)
                qk_group(wq_bf, qT, 0)
                x_chain_store(8)
                x_chain_store(9)
                qk_group(wq_bf, qT, 1)
                x_chain_store(10)
                x_chain_store(11)
                qk_group(wk_bf, kT, 0)
                x_chain_store(12)
                x_chain_store(13)
                qk_group(wk_bf, kT, 1)
                x_chain_store(14)
                x_chain_store(15)
                xbar_batch(2)
                # Early scores for i-spans 0-1: fills the PE bubble while
                # the ts2/ts3 xbar transposes complete.
                eidx = 0
                for s in range(2):
                    for jb in range(4 * s + 4):
                        i0 = max(s * 512, jb * P)
                        L = (s + 1) * 512 - i0
                        score_exp(jb, i0, L, etE[:, eidx, 0:L])
                        eidx += 1
                xbar_batch(3)
                qk_group(wq_bf, qT, 2)
                qk_group(wk_bf, kT, 2)
                qk_group(wq_bf, qT, 3)
                qk_group(wk_bf, kT, 3)

                # v: out[t(128), e(512)] = sum_d xT[d, t]-stat @ W[d, e]
                for db in range(DBLK):
                    w_chain(Wv, wv_bf, db, dma_engine=nc.gpsimd)
                for tb in range(TB):
                    for es in range(NES):
                        ps = psum.tile([P, 512], F32, tag="big")
                        for db in range(DBLK):
                            nc.tensor.matmul(
                                ps,
                                lhsT=xT[:, db, tb * P:(tb + 1) * P],
                                rhs=wv_bf[:, db, es * 512:(es + 1) * 512],
                                start=(db == 0),
                                stop=(db == DBLK - 1),
                            )
                        nc.any.tensor_copy(vsb[:, tb, es * 512:(es + 1) * 512], ps)

            # ================= Phase C+D: attention =====================
            with tc.tile_pool(name="etp", bufs=16) as etp, \
                 tc.tile_pool(name="outp", bufs=4) as outp, \
                 tc.tile_pool(name="rsp", bufs=4) as rsp:
                eidx = 0
                for s in range(NTS):
                    # --- scores + exp for i-span s, all jb <= 4s+3 ---
                    # (spans 0-1 were already computed inside phase B; see
                    # the early-scores fill)
                    et_tiles = []
                    et_i0 = []
                    for jb in range(4 * s + 4):
                        i0 = max(s * 512, jb * P)
                        L = (s + 1) * 512 - i0
                        if s < 2:
                            et = etE[:, eidx, :]
                            eidx += 1
                        else:
                            et = etp.tile([P, 512], BF16, tag="et")
                            score_exp(jb, i0, L, et[:, 0:L])
                        et_tiles.append(et)
                        et_i0.append(i0)

                    # --- AV + rowsums for the 4 i-blocks in span s ---
                    for ib in range(4 * s, 4 * s + 4):
                        ps0 = psum.tile([P, 512], F32, tag="big")
                        ps1 = psum.tile([P, 512], F32, tag="big")
                        pss = psum.tile([P, 1], F32, tag="small", bufs=2)
                        for jb in range(ib + 1):
                            off = ib * P - et_i0[jb]
                            lhsT = et_tiles[jb][:, off:off + P]
                            first = jb == 0
                            last = jb == ib
                            nc.tensor.matmul(
                                ps0, lhsT=lhsT, rhs=vsb[:, jb, 0:512],
                                start=first, stop=last,
                            )
                            nc.tensor.matmul(
                                ps1, lhsT=lhsT, rhs=vsb[:, jb, 512:1024],
                                start=first, stop=last,
                            )
                            nc.tensor.matmul(
                                pss, lhsT=lhsT, rhs=ones,
                                start=first, stop=last,
                            )
                        rsum = rsp.tile([P, 1], F32)
                        nc.vector.reciprocal(rsum, pss)
                        for es, ps in ((0, ps0), (1, ps1)):
                            ob = outp.tile([P, 512], F32)
                            nc.vector.tensor_scalar_mul(ob, ps, rsum)
                            nc.sync.dma_start(
                                out=out[ib * P:(ib + 1) * P,
                                        es * 512:(es + 1) * 512],
                                in_=ob,
                            )
    return nc


_NC_CACHE = None


def _get_nc():
    global _NC_CACHE
    if _NC_CACHE is None:
        nc = bass.Bass(
            "TRN2", target_bir_lowering=False, debug=False, num_devices=1
        )
        _emit(nc)
        _split_multi_waits(nc)
        _NC_CACHE = nc
    return _NC_CACHE


def kernel(x, Wq, Wk, Wv):
    assert x.shape == (B, T, D), x.shape
    nc = _get_nc()
    Wq = np.ascontiguousarray(Wq, dtype=np.float32)
    Wk = np.ascontiguousarray(Wk, dtype=np.float32)
    Wv = np.ascontiguousarray(Wv, dtype=np.float32)
    in_maps = [
        {
            "x": np.ascontiguousarray(x[b], dtype=np.float32),
            "Wq": Wq,
            "Wk": Wk,
            "Wv": Wv,
        }
        for b in range(B)
    ]
    res = run_bass_kernel_spmd(nc, in_maps, core_ids=list(range(B)))
    out = np.stack([res.results[b]["out"] for b in range(B)], axis=0)
    kernel.last_exec_time_ns = res.exec_time_ns
    return out
